# revision 1
# baseline (speedup 1.0000x reference)
"""HMM forward (negative log-marginal) on 8 TRN2 NeuronCores.

Algorithm: the log-space recurrence
    alpha_t[b,j] = obs_t[b,j] + LSE_i(alpha_{t-1}[b,i] + T_log[j,i])
is run in linear space with a constant per-step rescale:
    aE_t[j,b] = exp(obs_t[j,b] + SHIFT) * sum_i W[i,j] * aE_{t-1}[i,b]
with W[i,j] = p(j|i) = exp(T_log[j,i]).  Each step is then a 512x512
matmul against the constant W plus one elementwise multiply -- no
per-step exp/log.  Final answer: -log p = 255*SHIFT - log(sum_j aE_255).

Sharding: data-parallel over batch (64 -> 8 per core).  W replicated;
per-core eobs slice is 2MB bf16 resident in SBUF.
Device layout is [z, batch]: z chunk of 128 on partitions, batch on the
free axis, so the matmul keeps W stationary (16 LDW+MM pairs per step)
and the output layout equals the input layout (no transposes).
"""

import numpy as np
import ml_dtypes

Z = 512
X = 10000
SEQ = 256
B = 64
NCORES = 8
BS = B // NCORES  # 8 batch per core
P = 128
ZC = Z // P  # 4 z-chunks
SHIFT = 9.2
FORCE_ORDER = False
TCH = 51  # eobs t-chunk (5 * 51 = 255)
NCH = (SEQ - 1) // TCH

_NC_CACHE = {}


def _build_nc():
    if "nc" in _NC_CACHE:
        return _NC_CACHE["nc"]
    from concourse import bacc
    import concourse.mybir as mybir
    import concourse.tile as tile

    bf16 = mybir.dt.bfloat16
    f32 = mybir.dt.float32

    nc = bacc.Bacc("TRN2", target_bir_lowering=False, debug=False,
                   num_devices=NCORES)

    w_d = nc.dram_tensor("w", [Z, Z], bf16, kind="ExternalInput")
    eobs_d = nc.dram_tensor("eobs", [P, SEQ - 1, ZC, BS], bf16,
                            kind="ExternalInput")
    ae0_d = nc.dram_tensor("ae0", [P, ZC, BS], bf16, kind="ExternalInput")
    out_d = nc.dram_tensor("out", [1, BS], f32, kind="ExternalOutput")

    from concourse.tile_rust import add_dep_helper

    with tile.TileContext(nc) as tc:
        with (
            tc.tile_pool(name="constp", bufs=1) as constp,
            tc.tile_pool(name="aep", bufs=2) as aep,
            tc.tile_pool(name="psp", bufs=2, space="PSUM") as psp,
            tc.tile_pool(name="finp", bufs=1) as finp,
        ):
            # Constant weights: w_sb[p, ic, j] = W[ic*128+p, j]
            w_sb = constp.tile([P, ZC, Z], bf16, name="w_sb")
            for ic in range(ZC):
                nc.sync.dma_start(out=w_sb[:, ic, :],
                                  in_=w_d[ic * P:(ic + 1) * P, :])

            ae_init = constp.tile([P, ZC, BS], bf16, name="ae_init")
            nc.sync.dma_start(out=ae_init[:], in_=ae0_d[:])

            ones_sb = constp.tile([P, 1], bf16, name="ones_sb")
            nc.vector.memset(ones_sb[:], 1.0)
            # Load the Ln table set early so the final log doesn't stall.
            scratch = finp.tile([P, 1], f32, name="scratch")
            nc.scalar.activation(scratch[:], ones_sb[:],
                                 mybir.ActivationFunctionType.Ln)

            eobs_sb = []
            for k in range(NCH):
                et = constp.tile([P, TCH, ZC, BS], bf16, name=f"eobs_{k}",
                                 tag=f"eobs_{k}")
                nc.sync.dma_start(out=et[:],
                                  in_=eobs_d[:, k * TCH:(k + 1) * TCH, :, :])
                eobs_sb.append(et)

            # MM slot order per step: pair A = groups {0,1} completes by
            # slot 8 (its DVE evacuation overlaps slots 9-16); pair B's
            # inputs (ic 2,3) are first consumed at slot 5, giving the
            # late-produced aeB an extra ~100ns of cross-step slack.
            ORDER = [(0, 0), (0, 1), (1, 0), (1, 1),
                     (0, 2), (0, 3), (1, 2), (1, 3),
                     (2, 0), (2, 1), (3, 0), (3, 1),
                     (2, 2), (2, 3), (3, 2), (3, 3)]
            prev = [ae_init[:, ic, :] for ic in range(ZC)]
            prev_mm = None
            for t in range(1, SEQ):
                k, toff = divmod(t - 1, TCH)
                # two 2-bank psum tiles: pair p holds groups 2p, 2p+1
                psA = psp.tile([P, 2, 512], f32, tag="psA", name=f"psA_{t}")
                psB = psp.tile([P, 2, 512], f32, tag="psB", name=f"psB_{t}")
                pspair = [psA, psB]
                for (jc, ic) in ORDER:
                    m = nc.tensor.matmul(
                        pspair[jc // 2][:, jc % 2, 0:BS],
                        w_sb[:, ic, jc * P:(jc + 1) * P],
                        prev[ic],
                        start=(ic == 0),
                        stop=(ic == ZC - 1),
                        skip_group_check=True,
                    )
                    if prev_mm is not None and FORCE_ORDER:
                        add_dep_helper(prev_mm, m.ins, sync=False,
                                       reason="mm-order")
                    prev_mm = m.ins
                aeA = aep.tile([P, 2, BS], bf16, tag="aeA", name=f"aeA_{t}")
                aeB = aep.tile([P, 2, BS], bf16, tag="aeB", name=f"aeB_{t}")
                nc.vector.tensor_mul(aeA[:], psA[:, :, 0:BS],
                                     eobs_sb[k][:, toff, 0:2, :])
                nc.vector.tensor_mul(aeB[:], psB[:, :, 0:BS],
                                     eobs_sb[k][:, toff, 2:4, :])
                prev = [aeA[:, 0, :], aeA[:, 1, :], aeB[:, 0, :], aeB[:, 1, :]]

            # Final: s[b] = sum_z aE_255[z, b] via ones-matmul, then
            # out = -(log s - 255*SHIFT).
            psf = psp.tile([1, BS], f32, tag="psA", name="ps_fin")
            for ic in range(ZC):
                nc.tensor.matmul(psf[:], ones_sb[:], prev[ic],
                                 start=(ic == 0), stop=(ic == ZC - 1))
            lg = finp.tile([1, BS], f32, name="lg")
            nc.scalar.activation(lg[:], psf[:],
                                 mybir.ActivationFunctionType.Ln)
            res = finp.tile([1, BS], f32, name="res")
            nc.vector.tensor_scalar(res[:], lg[:], -1.0,
                                    float(SHIFT * (SEQ - 1)),
                                    mybir.AluOpType.mult,
                                    mybir.AluOpType.add)
            nc.sync.dma_start(out=out_d[:], in_=res[:])

    nc.compile()
    _NC_CACHE["nc"] = nc
    return nc


def _log_softmax64(x, axis):
    x = np.asarray(x, np.float64)
    m = x.max(axis=axis, keepdims=True)
    return x - m - np.log(np.exp(x - m).sum(axis=axis, keepdims=True))


def host_prep(input_ids, T, pi, emit):
    """Numpy prep: normalize params, gather per-step emissions, shard."""
    ids = np.asarray(input_ids).astype(np.int64)
    T_log = _log_softmax64(T, 0)
    pi_log = _log_softmax64(pi, 0)
    emit_log = _log_softmax64(emit, 0)
    W = np.exp(T_log).T  # [i, j] = p(j|i)
    obs = emit_log[ids]  # [256, 64, 512]
    eobs = np.exp(obs[1:] + SHIFT)  # [255, 64, 512]
    ae0 = np.exp(obs[0] + pi_log[None, :])  # [64, 512]

    bf = ml_dtypes.bfloat16
    w_dev = np.ascontiguousarray(W.astype(bf))
    in_maps = []
    for c in range(NCORES):
        bsl = slice(c * BS, (c + 1) * BS)
        e = eobs[:, bsl, :].reshape(SEQ - 1, BS, ZC, P)
        e = np.ascontiguousarray(e.transpose(3, 0, 2, 1).astype(bf))
        a = ae0[bsl, :].reshape(BS, ZC, P)
        a = np.ascontiguousarray(a.transpose(2, 1, 0).astype(bf))
        in_maps.append({"w": w_dev, "eobs": e, "ae0": a})
    return in_maps


def kernel(input_ids, T, pi, emit, _trace=False):
    from concourse.bass_utils import run_bass_kernel_spmd

    nc = _build_nc()
    in_maps = host_prep(input_ids, T, pi, emit)
    r = run_bass_kernel_spmd(nc, in_maps, core_ids=list(range(NCORES)),
                             trace=_trace)
    out = np.concatenate([r.results[c]["out"][0] for c in range(NCORES)])
    if _trace:
        kernel.last_results = r
    return out.astype(np.float32)



# revision 5
# speedup vs baseline: 1.0071x; 1.0071x over previous
"""HMM forward (negative log-marginal) on 8 TRN2 NeuronCores.

Algorithm: the log-space recurrence
    alpha_t[b,j] = obs_t[b,j] + LSE_i(alpha_{t-1}[b,i] + T_log[j,i])
is run in linear space with a constant per-step rescale:
    aE_t[j,b] = exp(obs_t[j,b] + SHIFT) * sum_i W[i,j] * aE_{t-1}[i,b]
with W[i,j] = p(j|i) = exp(T_log[j,i]).  Each step is then a 512x512
matmul against the constant W plus one elementwise multiply -- no
per-step exp/log.  Final answer: -log p = 255*SHIFT - log(sum_j aE_255).

Sharding: data-parallel over batch (64 -> 8 per core).  W replicated;
per-core eobs slice is 2MB bf16 resident in SBUF.
Device layout is [z, batch]: z chunk of 128 on partitions, batch on the
free axis, so the matmul keeps W stationary (16 LDW+MM pairs per step)
and the output layout equals the input layout (no transposes).
"""

import numpy as np
import ml_dtypes

Z = 512
X = 10000
SEQ = 256
B = 64
NCORES = 8
BS = B // NCORES  # 8 batch per core
P = 128
ZC = Z // P  # 4 z-chunks
SHIFT = 9.2
FORCE_ORDER = False
TCH = 51  # eobs t-chunk (5 * 51 = 255)
NCH = (SEQ - 1) // TCH

_NC_CACHE = {}


def _build_nc():
    if "nc" in _NC_CACHE:
        return _NC_CACHE["nc"]
    from concourse import bacc
    import concourse.mybir as mybir
    import concourse.tile as tile

    bf16 = mybir.dt.bfloat16
    fp8 = mybir.dt.float8e4
    f32 = mybir.dt.float32

    nc = bacc.Bacc("TRN2", target_bir_lowering=False, debug=False,
                   num_devices=NCORES)

    w_d = nc.dram_tensor("w", [Z, Z], fp8, kind="ExternalInput")
    eobs_d = nc.dram_tensor("eobs", [P, SEQ - 1, ZC, BS], bf16,
                            kind="ExternalInput")
    ae0_d = nc.dram_tensor("ae0", [P, ZC, BS], bf16, kind="ExternalInput")
    out_d = nc.dram_tensor("out", [1, BS], f32, kind="ExternalOutput")

    from concourse.tile_rust import add_dep_helper

    with tile.TileContext(nc) as tc:
        with (
            tc.tile_pool(name="constp", bufs=1) as constp,
            tc.tile_pool(name="aep", bufs=2) as aep,
            tc.tile_pool(name="psp", bufs=2, space="PSUM") as psp,
            tc.tile_pool(name="finp", bufs=1) as finp,
        ):
            # Constant weights: w_sb[p, ic, j] = W[ic*128+p, j]
            w_sb = constp.tile([P, ZC, Z], fp8, name="w_sb")
            for ic in range(ZC):
                nc.sync.dma_start(out=w_sb[:, ic, :],
                                  in_=w_d[ic * P:(ic + 1) * P, :])

            ae_init = constp.tile([P, ZC, BS], bf16, name="ae_init")
            nc.sync.dma_start(out=ae_init[:], in_=ae0_d[:])

            ones_sb = constp.tile([P, 1], bf16, name="ones_sb")
            nc.vector.memset(ones_sb[:], 1.0)
            # Load the Ln table set early so the final log doesn't stall.
            scratch = finp.tile([P, 1], f32, name="scratch")
            nc.scalar.activation(scratch[:], ones_sb[:],
                                 mybir.ActivationFunctionType.Ln)

            eobs_sb = []
            for k in range(NCH):
                et = constp.tile([P, TCH, ZC, BS], bf16, name=f"eobs_{k}",
                                 tag=f"eobs_{k}")
                nc.sync.dma_start(out=et[:],
                                  in_=eobs_d[:, k * TCH:(k + 1) * TCH, :, :])
                eobs_sb.append(et)

            # MM slot order per step: pair A = groups {0,1} completes by
            # slot 8 (its DVE evacuation overlaps slots 9-16); pair B's
            # inputs (ic 2,3) are first consumed at slot 5, giving the
            # late-produced aeB an extra ~100ns of cross-step slack.
            ORDER = [(0, 0), (0, 1), (1, 0), (1, 1),
                     (0, 2), (0, 3), (1, 2), (1, 3),
                     (2, 0), (2, 1), (3, 0), (3, 1),
                     (2, 2), (2, 3), (3, 2), (3, 3)]
            prev = [ae_init[:, ic, :] for ic in range(ZC)]
            prev_mm = None
            for t in range(1, SEQ):
                k, toff = divmod(t - 1, TCH)
                # two 2-bank psum tiles: pair p holds groups 2p, 2p+1
                psA = psp.tile([P, 2, 512], f32, tag="psA", name=f"psA_{t}")
                psB = psp.tile([P, 2, 512], f32, tag="psB", name=f"psB_{t}")
                pspair = [psA, psB]
                for (jc, ic) in ORDER:
                    m = nc.tensor.matmul(
                        pspair[jc // 2][:, jc % 2, 0:BS],
                        w_sb[:, ic, jc * P:(jc + 1) * P],
                        prev[ic],
                        start=(ic == 0),
                        stop=(ic == ZC - 1),
                        skip_group_check=True,
                    )
                    if prev_mm is not None and FORCE_ORDER:
                        add_dep_helper(prev_mm, m.ins, sync=False,
                                       reason="mm-order")
                    prev_mm = m.ins
                aeA = aep.tile([P, 2, BS], bf16, tag="aeA", name=f"aeA_{t}")
                aeB = aep.tile([P, 2, BS], bf16, tag="aeB", name=f"aeB_{t}")
                nc.vector.tensor_mul(aeA[:], psA[:, :, 0:BS],
                                     eobs_sb[k][:, toff, 0:2, :])
                nc.vector.tensor_mul(aeB[:], psB[:, :, 0:BS],
                                     eobs_sb[k][:, toff, 2:4, :])
                prev = [aeA[:, 0, :], aeA[:, 1, :], aeB[:, 0, :], aeB[:, 1, :]]

            # Final: s[b] = sum_z aE_255[z, b] via ones-matmul, then
            # out = -(log s - 255*SHIFT).
            psf = psp.tile([1, BS], f32, tag="psA", name="ps_fin")
            for ic in range(ZC):
                nc.tensor.matmul(psf[:], ones_sb[:], prev[ic],
                                 start=(ic == 0), stop=(ic == ZC - 1))
            lg = finp.tile([1, BS], f32, name="lg")
            nc.scalar.activation(lg[:], psf[:],
                                 mybir.ActivationFunctionType.Ln)
            res = finp.tile([1, BS], f32, name="res")
            nc.vector.tensor_scalar(res[:], lg[:], -1.0,
                                    float(SHIFT * (SEQ - 1)),
                                    mybir.AluOpType.mult,
                                    mybir.AluOpType.add)
            nc.sync.dma_start(out=out_d[:], in_=res[:])

    nc.compile()
    _NC_CACHE["nc"] = nc
    return nc


def _log_softmax64(x, axis):
    x = np.asarray(x, np.float64)
    m = x.max(axis=axis, keepdims=True)
    return x - m - np.log(np.exp(x - m).sum(axis=axis, keepdims=True))


def host_prep(input_ids, T, pi, emit):
    """Numpy prep: normalize params, gather per-step emissions, shard."""
    ids = np.asarray(input_ids).astype(np.int64)
    T_log = _log_softmax64(T, 0)
    pi_log = _log_softmax64(pi, 0)
    emit_log = _log_softmax64(emit, 0)
    W = np.exp(T_log).T  # [i, j] = p(j|i)
    # fp8 weights: scale W by a power of two so max lands near ~60 (well
    # under the trn fp8e4 max of 240); fold 1/s into the per-step eobs so
    # the on-device 255*SHIFT constant stays exact.
    s = 2.0 ** np.floor(np.log2(60.0 / W.max()))
    obs = emit_log[ids]  # [256, 64, 512]
    eobs = np.exp(obs[1:] + SHIFT) / s  # [255, 64, 512]
    ae0 = np.exp(obs[0] + pi_log[None, :])  # [64, 512]

    bf = ml_dtypes.bfloat16
    w_dev = np.ascontiguousarray((W * s).astype(ml_dtypes.float8_e4m3))
    in_maps = []
    for c in range(NCORES):
        bsl = slice(c * BS, (c + 1) * BS)
        e = eobs[:, bsl, :].reshape(SEQ - 1, BS, ZC, P)
        e = np.ascontiguousarray(e.transpose(3, 0, 2, 1).astype(bf))
        a = ae0[bsl, :].reshape(BS, ZC, P)
        a = np.ascontiguousarray(a.transpose(2, 1, 0).astype(bf))
        in_maps.append({"w": w_dev, "eobs": e, "ae0": a})
    return in_maps


def kernel(input_ids, T, pi, emit, _trace=False):
    from concourse.bass_utils import run_bass_kernel_spmd

    nc = _build_nc()
    in_maps = host_prep(input_ids, T, pi, emit)
    r = run_bass_kernel_spmd(nc, in_maps, core_ids=list(range(NCORES)),
                             trace=_trace)
    out = np.concatenate([r.results[c]["out"][0] for c in range(NCORES)])
    if _trace:
        kernel.last_results = r
    return out.astype(np.float32)



# revision 8
# speedup vs baseline: 1.0853x; 1.0776x over previous
"""HMM forward (negative log-marginal) on 8 TRN2 NeuronCores.

Algorithm: the log-space recurrence
    alpha_t[b,j] = obs_t[b,j] + LSE_i(alpha_{t-1}[b,i] + T_log[j,i])
is run in linear space with a constant per-step rescale:
    aE_t[j,b] = exp(obs_t[j,b] + SHIFT) * sum_i W[i,j] * aE_{t-1}[i,b]
with W[i,j] = p(j|i) = exp(T_log[j,i]).  Each step is then a 512x512
matmul against the constant W plus one elementwise multiply -- no
per-step exp/log.  Final answer: -log p = 255*SHIFT - log(sum_j aE_255).

Sharding: data-parallel over batch (64 -> 8 per core).  W replicated;
per-core eobs slice is 2MB bf16 resident in SBUF.
Device layout is [z, batch]: z chunk of 128 on partitions, batch on the
free axis, so the matmul keeps W stationary (16 LDW+MM pairs per step)
and the output layout equals the input layout (no transposes).
"""

import numpy as np
import ml_dtypes

Z = 512
X = 10000
SEQ = 256
B = 64
NCORES = 8
BS = B // NCORES  # 8 batch per core
P = 128
ZC = Z // P  # 4 z-chunks
SHIFT = 9.2
FORCE_ORDER = False
TCH = 51  # eobs t-chunk (5 * 51 = 255)
NCH = (SEQ - 1) // TCH

_NC_CACHE = {}


def _build_nc():
    if "nc" in _NC_CACHE:
        return _NC_CACHE["nc"]
    from concourse import bacc
    import concourse.mybir as mybir
    import concourse.tile as tile

    bf16 = mybir.dt.bfloat16
    fp8 = mybir.dt.float8e4
    f32 = mybir.dt.float32

    nc = bacc.Bacc("TRN2", target_bir_lowering=False, debug=False,
                   num_devices=NCORES)

    w_d = nc.dram_tensor("w", [Z, Z], fp8, kind="ExternalInput")
    eobs_d = nc.dram_tensor("eobs", [P, SEQ - 1, ZC, BS], bf16,
                            kind="ExternalInput")
    ae0_d = nc.dram_tensor("ae0", [P, ZC, BS], bf16, kind="ExternalInput")
    out_d = nc.dram_tensor("out", [1, BS], f32, kind="ExternalOutput")

    from concourse.tile_rust import add_dep_helper

    with tile.TileContext(nc) as tc:
        with (
            tc.tile_pool(name="constp", bufs=1) as constp,
            tc.tile_pool(name="aep", bufs=2) as aep,
            tc.tile_pool(name="psp", bufs=2, space="PSUM") as psp,
            tc.tile_pool(name="finp", bufs=1) as finp,
        ):
            # Constant weights: w_sb[p, ic, j] = W[ic*128+p, j]
            w_sb = constp.tile([P, ZC, Z], fp8, name="w_sb")
            for ic in range(ZC):
                nc.sync.dma_start(out=w_sb[:, ic, :],
                                  in_=w_d[ic * P:(ic + 1) * P, :])

            ae_init = constp.tile([P, ZC, BS], bf16, name="ae_init")
            nc.sync.dma_start(out=ae_init[:], in_=ae0_d[:])

            ones_sb = constp.tile([P, 1], bf16, name="ones_sb")
            nc.vector.memset(ones_sb[:], 1.0)
            # Load the Ln table set early so the final log doesn't stall.
            scratch = finp.tile([P, 1], f32, name="scratch")
            nc.scalar.activation(scratch[:], ones_sb[:],
                                 mybir.ActivationFunctionType.Ln)

            eobs_sb = []
            for k in range(NCH):
                et = constp.tile([P, TCH, ZC, BS], bf16, name=f"eobs_{k}",
                                 tag=f"eobs_{k}")
                nc.sync.dma_start(out=et[:],
                                  in_=eobs_d[:, k * TCH:(k + 1) * TCH, :, :])
                eobs_sb.append(et)

            # MM slot order per step, found by discrete-event search over the
            # measured pipeline model (LDW+MM pair cadence ~26.5ns, MM drain
            # ~167ns, DVE TT ~172ns, sem latencies ~11/58ns).  Groups {0,1}
            # complete mid-stream so TT-A overlaps the remaining MMs; reads
            # of late-evacuated chunks sit late in the next step's stream.
            ORDER = [(1, 0), (0, 1), (1, 1), (2, 1),
                     (0, 0), (3, 0), (1, 2), (0, 2),
                     (0, 3), (1, 3), (2, 3), (3, 2),
                     (2, 2), (2, 0), (3, 3), (3, 1)]
            # start/stop flags per slot: first/last occurrence of each group
            first_slot = {}
            last_slot = {}
            for s, (jc, ic) in enumerate(ORDER):
                first_slot.setdefault(jc, s)
                last_slot[jc] = s
            prev = [ae_init[:, ic, :] for ic in range(ZC)]
            prev_mm = None
            for t in range(1, SEQ):
                k, toff = divmod(t - 1, TCH)
                # two 2-bank psum tiles: pair p holds groups 2p, 2p+1
                psA = psp.tile([P, 2, 512], f32, tag="psA", name=f"psA_{t}")
                psB = psp.tile([P, 2, 512], f32, tag="psB", name=f"psB_{t}")
                pspair = [psA, psB]
                for s, (jc, ic) in enumerate(ORDER):
                    m = nc.tensor.matmul(
                        pspair[jc // 2][:, jc % 2, 0:BS],
                        w_sb[:, ic, jc * P:(jc + 1) * P],
                        prev[ic],
                        start=(s == first_slot[jc]),
                        stop=(s == last_slot[jc]),
                        skip_group_check=True,
                    )
                    if prev_mm is not None:
                        # m depends on prev_mm (arg order: from depends on to)
                        add_dep_helper(m.ins, prev_mm, sync=False,
                                       reason="mm-order")
                    prev_mm = m.ins
                aeA = aep.tile([P, 2, BS], bf16, tag="aeA", name=f"aeA_{t}")
                aeB = aep.tile([P, 2, BS], bf16, tag="aeB", name=f"aeB_{t}")
                ttA = nc.vector.tensor_mul(aeA[:], psA[:, :, 0:BS],
                                           eobs_sb[k][:, toff, 0:2, :])
                ttB = nc.vector.tensor_mul(aeB[:], psB[:, :, 0:BS],
                                           eobs_sb[k][:, toff, 2:4, :])
                add_dep_helper(ttB.ins, ttA.ins, sync=False, reason="tt-order")
                prev = [aeA[:, 0, :], aeA[:, 1, :], aeB[:, 0, :], aeB[:, 1, :]]

            # Final: s[b] = sum_z aE_255[z, b] via ones-matmul, then
            # out = -(log s - 255*SHIFT).
            psf = psp.tile([1, BS], f32, tag="psA", name="ps_fin")
            for ic in range(ZC):
                nc.tensor.matmul(psf[:], ones_sb[:], prev[ic],
                                 start=(ic == 0), stop=(ic == ZC - 1))
            lg = finp.tile([1, BS], f32, name="lg")
            nc.scalar.activation(lg[:], psf[:],
                                 mybir.ActivationFunctionType.Ln)
            res = finp.tile([1, BS], f32, name="res")
            nc.vector.tensor_scalar(res[:], lg[:], -1.0,
                                    float(SHIFT * (SEQ - 1)),
                                    mybir.AluOpType.mult,
                                    mybir.AluOpType.add)
            nc.sync.dma_start(out=out_d[:], in_=res[:])

    nc.compile()
    _NC_CACHE["nc"] = nc
    return nc


def _log_softmax64(x, axis):
    x = np.asarray(x, np.float64)
    m = x.max(axis=axis, keepdims=True)
    return x - m - np.log(np.exp(x - m).sum(axis=axis, keepdims=True))


def host_prep(input_ids, T, pi, emit):
    """Numpy prep: normalize params, gather per-step emissions, shard."""
    ids = np.asarray(input_ids).astype(np.int64)
    T_log = _log_softmax64(T, 0)
    pi_log = _log_softmax64(pi, 0)
    emit_log = _log_softmax64(emit, 0)
    W = np.exp(T_log).T  # [i, j] = p(j|i)
    # fp8 weights: scale W by a power of two so max lands near ~60 (well
    # under the trn fp8e4 max of 240); fold 1/s into the per-step eobs so
    # the on-device 255*SHIFT constant stays exact.
    s = 2.0 ** np.floor(np.log2(60.0 / W.max()))
    obs = emit_log[ids]  # [256, 64, 512]
    eobs = np.exp(obs[1:] + SHIFT) / s  # [255, 64, 512]
    ae0 = np.exp(obs[0] + pi_log[None, :])  # [64, 512]

    bf = ml_dtypes.bfloat16
    w_dev = np.ascontiguousarray((W * s).astype(ml_dtypes.float8_e4m3))
    in_maps = []
    for c in range(NCORES):
        bsl = slice(c * BS, (c + 1) * BS)
        e = eobs[:, bsl, :].reshape(SEQ - 1, BS, ZC, P)
        e = np.ascontiguousarray(e.transpose(3, 0, 2, 1).astype(bf))
        a = ae0[bsl, :].reshape(BS, ZC, P)
        a = np.ascontiguousarray(a.transpose(2, 1, 0).astype(bf))
        in_maps.append({"w": w_dev, "eobs": e, "ae0": a})
    return in_maps


def kernel(input_ids, T, pi, emit, _trace=False):
    from concourse.bass_utils import run_bass_kernel_spmd

    nc = _build_nc()
    in_maps = host_prep(input_ids, T, pi, emit)
    r = run_bass_kernel_spmd(nc, in_maps, core_ids=list(range(NCORES)),
                             trace=_trace)
    out = np.concatenate([r.results[c]["out"][0] for c in range(NCORES)])
    if _trace:
        kernel.last_results = r
    return out.astype(np.float32)



# revision 14
# speedup vs baseline: 1.1000x; 1.0136x over previous
"""HMM forward (negative log-marginal) on 8 TRN2 NeuronCores.

Algorithm: the log-space recurrence
    alpha_t[b,j] = obs_t[b,j] + LSE_i(alpha_{t-1}[b,i] + T_log[j,i])
is run in linear space with a constant per-step rescale:
    aE_t[j,b] = exp(obs_t[j,b] + SHIFT) * sum_i W[i,j] * aE_{t-1}[i,b]
with W[i,j] = p(j|i) = exp(T_log[j,i]).  Each step is then a 512x512
matmul against the constant W plus one elementwise multiply -- no
per-step exp/log.  Final answer: -log p = 255*SHIFT - log(sum_j aE_255).

Sharding: data-parallel over batch (64 -> 8 per core).  W replicated;
per-core eobs slice is 2MB bf16 resident in SBUF.
Device layout is [z, batch]: z chunk of 128 on partitions, batch on the
free axis, so the matmul keeps W stationary (16 LDW+MM pairs per step)
and the output layout equals the input layout (no transposes).
"""

import numpy as np
import ml_dtypes

Z = 512
X = 10000
SEQ = 256
B = 64
NCORES = 8
BS = B // NCORES  # 8 batch per core
P = 128
ZC = Z // P  # 4 z-chunks
SHIFT = 9.2
T0 = 16  # first eobs chunk: small so the first DVE multiply ungates early

_NC_CACHE = {}


def _build_nc():
    if "nc" in _NC_CACHE:
        return _NC_CACHE["nc"]
    from concourse import bacc
    import concourse.mybir as mybir
    import concourse.tile as tile

    bf16 = mybir.dt.bfloat16
    fp8 = mybir.dt.float8e4
    f32 = mybir.dt.float32

    nc = bacc.Bacc("TRN2", target_bir_lowering=False, debug=False,
                   num_devices=NCORES)

    # w already in device layout [p, ic, j] (host shuffles) -> one DMA
    w_d = nc.dram_tensor("w", [P, ZC * Z], fp8, kind="ExternalInput")
    eobs_d = nc.dram_tensor("eobs", [P, SEQ - 1, ZC, BS], bf16,
                            kind="ExternalInput")
    ae0_d = nc.dram_tensor("ae0", [P, ZC, BS], bf16, kind="ExternalInput")
    out_d = nc.dram_tensor("out", [1, BS], f32, kind="ExternalOutput")

    from concourse.tile_rust import add_dep_helper

    with tile.TileContext(nc) as tc:
        with (
            tc.tile_pool(name="constp", bufs=1) as constp,
            tc.tile_pool(name="aep", bufs=2) as aep,
            tc.tile_pool(name="psp", bufs=2, space="PSUM") as psp,
            tc.tile_pool(name="finp", bufs=1) as finp,
        ):
            # Constant weights: w_sb[p, ic, j] = W[ic*128+p, j] -- one DMA
            w_sb = constp.tile([P, ZC, Z], fp8, name="w_sb")
            nc.sync.dma_start(out=w_sb[:], in_=w_d[:])

            ae_init = constp.tile([P, ZC, BS], bf16, name="ae_init")
            nc.sync.dma_start(out=ae_init[:], in_=ae0_d[:])

            # eobs in two tiles: small head chunk first so step 1's DVE
            # multiply ungates as early as possible, bulk streams under
            # compute.
            eobs0_sb = constp.tile([P, T0, ZC, BS], bf16, name="eobs0")
            nc.sync.dma_start(out=eobs0_sb[:], in_=eobs_d[:, 0:T0, :, :])
            eobsR_sb = constp.tile([P, SEQ - 1 - T0, ZC, BS], bf16,
                                   name="eobsR")
            nc.sync.dma_start(out=eobsR_sb[:], in_=eobs_d[:, T0:, :, :])

            ones_sb = constp.tile([P, 1], bf16, name="ones_sb")
            nc.vector.memset(ones_sb[:], 1.0)
            # Load the Ln table set early so the final log doesn't stall.
            scratch = finp.tile([P, 1], f32, name="scratch")
            nc.scalar.activation(scratch[:], ones_sb[:],
                                 mybir.ActivationFunctionType.Ln)

            # MM slot order per step, found by discrete-event search over the
            # measured pipeline model (LDW+MM pair cadence ~26.5ns, MM drain
            # ~167ns, DVE TT ~172ns, sem latencies ~11/58ns).  Groups {0,1}
            # complete mid-stream so TT-A overlaps the remaining MMs; reads
            # of late-evacuated chunks sit late in the next step's stream.
            ORDER = [(1, 0), (0, 1), (1, 1), (2, 1),
                     (0, 0), (3, 0), (1, 2), (0, 2),
                     (0, 3), (1, 3), (2, 3), (3, 2),
                     (2, 2), (2, 0), (3, 3), (3, 1)]
            # start/stop flags per slot: first/last occurrence of each group
            first_slot = {}
            last_slot = {}
            for s, (jc, ic) in enumerate(ORDER):
                first_slot.setdefault(jc, s)
                last_slot[jc] = s
            prev = [ae_init[:, ic, :] for ic in range(ZC)]
            prev_mm = None
            for t in range(1, SEQ):
                if t - 1 < T0:
                    esb, toff = eobs0_sb, t - 1
                else:
                    esb, toff = eobsR_sb, t - 1 - T0
                # two 2-bank psum tiles: pair p holds groups 2p, 2p+1
                psA = psp.tile([P, 2, 512], f32, tag="psA", name=f"psA_{t}")
                psB = psp.tile([P, 2, 512], f32, tag="psB", name=f"psB_{t}")
                pspair = [psA, psB]
                for s, (jc, ic) in enumerate(ORDER):
                    m = nc.tensor.matmul(
                        pspair[jc // 2][:, jc % 2, 0:BS],
                        w_sb[:, ic, jc * P:(jc + 1) * P],
                        prev[ic],
                        start=(s == first_slot[jc]),
                        stop=(s == last_slot[jc]),
                        skip_group_check=True,
                    )
                    if prev_mm is not None:
                        # m depends on prev_mm (arg order: from depends on to)
                        add_dep_helper(m.ins, prev_mm, sync=False,
                                       reason="mm-order")
                    prev_mm = m.ins
                aeA = aep.tile([P, 2, BS], bf16, tag="aeA", name=f"aeA_{t}")
                aeB = aep.tile([P, 2, BS], bf16, tag="aeB", name=f"aeB_{t}")
                ttA = nc.vector.tensor_mul(aeA[:], psA[:, :, 0:BS],
                                           esb[:, toff, 0:2, :])
                ttB = nc.vector.tensor_mul(aeB[:], psB[:, :, 0:BS],
                                           esb[:, toff, 2:4, :])
                add_dep_helper(ttB.ins, ttA.ins, sync=False, reason="tt-order")
                prev = [aeA[:, 0, :], aeA[:, 1, :], aeB[:, 0, :], aeB[:, 1, :]]

            # Final: s[b] = sum_z aE_255[z, b] via ones-matmul, then
            # out = -(log s - 255*SHIFT).
            psf = psp.tile([1, BS], f32, tag="psA", name="ps_fin")
            for ic in range(ZC):
                nc.tensor.matmul(psf[:], ones_sb[:], prev[ic],
                                 start=(ic == 0), stop=(ic == ZC - 1))
            lg = finp.tile([1, BS], f32, name="lg")
            nc.scalar.activation(lg[:], psf[:],
                                 mybir.ActivationFunctionType.Ln)
            res = finp.tile([1, BS], f32, name="res")
            nc.vector.tensor_scalar(res[:], lg[:], -1.0,
                                    float(SHIFT * (SEQ - 1)),
                                    mybir.AluOpType.mult,
                                    mybir.AluOpType.add)
            nc.sync.dma_start(out=out_d[:], in_=res[:])

    nc.compile()
    _NC_CACHE["nc"] = nc
    return nc


def _log_softmax64(x, axis):
    x = np.asarray(x, np.float64)
    m = x.max(axis=axis, keepdims=True)
    return x - m - np.log(np.exp(x - m).sum(axis=axis, keepdims=True))


def host_prep(input_ids, T, pi, emit):
    """Numpy prep: normalize params, gather per-step emissions, shard."""
    ids = np.asarray(input_ids).astype(np.int64)
    T_log = _log_softmax64(T, 0)
    pi_log = _log_softmax64(pi, 0)
    emit_log = _log_softmax64(emit, 0)
    W = np.exp(T_log).T  # [i, j] = p(j|i)
    # fp8 weights: scale W by a power of two so max lands near ~60 (well
    # under the trn fp8e4 max of 240); fold 1/s into the per-step eobs so
    # the on-device 255*SHIFT constant stays exact.
    s = 2.0 ** np.floor(np.log2(60.0 / W.max()))
    obs = emit_log[ids]  # [256, 64, 512]
    eobs = np.exp(obs[1:] + SHIFT) / s  # [255, 64, 512]
    ae0 = np.exp(obs[0] + pi_log[None, :])  # [64, 512]

    bf = ml_dtypes.bfloat16
    # device weight layout [p, ic*512 + j] = W'[ic*128+p, j]
    w_dev = np.ascontiguousarray(
        (W * s).astype(ml_dtypes.float8_e4m3)
        .reshape(ZC, P, Z).transpose(1, 0, 2).reshape(P, ZC * Z))
    in_maps = []
    for c in range(NCORES):
        bsl = slice(c * BS, (c + 1) * BS)
        e = eobs[:, bsl, :].reshape(SEQ - 1, BS, ZC, P)
        e = np.ascontiguousarray(e.transpose(3, 0, 2, 1).astype(bf))
        a = ae0[bsl, :].reshape(BS, ZC, P)
        a = np.ascontiguousarray(a.transpose(2, 1, 0).astype(bf))
        in_maps.append({"w": w_dev, "eobs": e, "ae0": a})
    return in_maps


def kernel(input_ids, T, pi, emit, _trace=False):
    from concourse.bass_utils import run_bass_kernel_spmd

    nc = _build_nc()
    in_maps = host_prep(input_ids, T, pi, emit)
    r = run_bass_kernel_spmd(nc, in_maps, core_ids=list(range(NCORES)),
                             trace=_trace)
    out = np.concatenate([r.results[c]["out"][0] for c in range(NCORES)])
    if _trace:
        kernel.last_results = r
    return out.astype(np.float32)

